# revision 38
# baseline (speedup 1.0000x reference)
"""Trainium2 Bass kernel for nn_MessageFunction (GNN message passing).

reference:
    edge_out = einsum('ben,em->bmn', e_vw, W_e) + b_e   # [B, 128, N]
    node_out = einsum('bfn,fm->bmn', h_w,  W_n) + b_n   # [B, 128, N]
    out      = relu(concat([edge_out, node_out], axis=1))  # [B, 256, N]

h_v is an unused input (dead in the reference) — never transferred.

Sharding: data-parallel over the node axis (last dim) across 8 cores,
weights/biases replicated. Each core handles 6250 nodes.

The whole pipeline runs in bf16 (inputs, weights, matmul, output), which
halves HBM traffic vs fp32: per-core 12.8 MB in + 12.8 MB out = 25.6 MB
(~61 us at the ~420 GB/s per-core HBM rate actually observed). Matmul
accumulation stays fp32 in PSUM; bias+ReLU read PSUM fp32 and round once
to bf16 on the way to SBUF. End-to-end rounding error ~2.9e-3 rel,
comfortably inside the 2e-2 gate. Host side converts fp32->bf16
(ml_dtypes, round-to-nearest-even) before upload and upcasts the bf16
result back to fp32.

Layout/schedule (all tuned against perfetto traces):
  - host folds batch into the node axis (e_all/h_all = [128, 4*6250])
    and prepends one 260-col const block (W_e|W_n|b_e|b_n, biases as
    fp32 bit patterns, bitcast on device) to the e stream: the consts
    ride in the FIRST e-load, so the first matmul is not pinned on a
    slow tiny const DMA (small DMAs take ~3.5 us to complete),
  - ~2k-col tiles = 4KB+ DMA lines; e loads + most h loads on the sync
    HWDGE ring, first 4 h loads on the scalar HWDGE ring so the two
    rings dispatch the ramp in parallel (~420 GB/s by 12 us),
  - matmul against resident 128x128 bf16 weights (K=128 contraction),
    <=512-col splits (one PSUM bank each, 8-bank rotation),
  - bias + ReLU fused: edge half on ScalarE (activation Relu w/ bias),
    node half on VectorE (tensor_scalar add+max) so the two engines
    run in parallel; both write bf16,
  - edge-half store on ACT's HWDGE ring (depends only on ACT's own
    output -> no cross-engine head-of-line blocking), node-half store
    on SWDGE (gpsimd otherwise idle); last batch tapered so the final
    store drain is short.

Known-bad variants (measured): h-loads via gpsimd SWDGE (slow ~165 GB/s
transfers stall the in-order PE pipeline), deferred stores on sync (its
queue is still draining the last loads), early small tiles (HBM is
load-saturated during the ramp anyway, they just slow dispatch).
"""

import numpy as np
import ml_dtypes

import concourse.bass as bass
import concourse.mybir as mybir
import concourse.tile as tile
from concourse import bacc
from concourse.bass_utils import run_bass_kernel_spmd

N_CORES = 8
B = 4
F = 128      # EDGE_F == NODE_F (contraction dim)
HALF = 128   # output channels per linear
N_NODES = 50000
NS = N_NODES // N_CORES       # 6250 nodes per core
T_MAX = 2176                  # SBUF tile capacity (cols)
CPAD = 2 * HALF + 4           # const columns prepended to the e stream

# Per-batch tile widths (sum 6250). 4KB+ DMA lines in bf16; the odd 106
# remainder (6250 mod 512) is folded into one 2154-wide tile so no DMA
# line drops below 1KB. Uniform-large tiles: the DMA ramp is limited by
# per-queue transfer pipelining, and small lead-in tiles only slow it.
_BODY = [2048, 2048, 2154]
# last batch tapered at the end: final tiles small so the store drain
# after the last load is short
_TILESL = [2154, 2048, 1024, 512, 512]
# first h-loads dispatched on the scalar HWDGE ring (its only early work
# is the 66KB const DMA): two queues dispatch the ramp in parallel, so
# ~4MB is in flight by ~12us instead of ~14.5us. Scalar's first ACT op
# needs the first matmul (~11us) anyway, so the extra dispatches ahead
# of it in scalar's FIFO don't delay compute.
_H_SCALAR = 4


def _mm_splits(width):
    # <=512 fp32 accumulators per matmul (one PSUM bank); near-uniform
    n = -(-width // 512)
    base, rem = divmod(width, n)
    return [base + (1 if i < rem else 0) for i in range(n)]

_BF16 = mybir.dt.bfloat16
_FP32 = mybir.dt.float32

_compiled = None


def _build():
    nc = bacc.Bacc(
        "TRN2",
        target_bir_lowering=False,
        debug=False,
        num_devices=N_CORES,
    )
    # host folds the batch dim into the node axis: e_all = [F, CPAD+B*NS]
    # with the constants packed into the leading CPAD cols (cols [0:128]=
    # W_e, [128:256]=W_n, then b_e and b_n as fp32 bit patterns in 2 bf16
    # slots each). The consts ride along in the FIRST e-load — no separate
    # const DMA, and the first matmul is not pinned on a slow tiny DMA.
    e_all = nc.dram_tensor("e_all", (F, CPAD + B * NS), _BF16, kind="ExternalInput").ap()
    h_all = nc.dram_tensor("h_all", (F, B * NS), _BF16, kind="ExternalInput").ap()
    out = nc.dram_tensor("out", (B, 2 * HALF, NS), _BF16, kind="ExternalOutput").ap()

    relu = mybir.ActivationFunctionType.Relu
    alu_add = mybir.AluOpType.add
    alu_max = mybir.AluOpType.max

    with tile.TileContext(nc) as tc:
        tiles = []
        for bb in range(B):
            n0 = 0
            for width in (_TILESL if bb == B - 1 else _BODY):
                tiles.append((bb, n0, width))
                n0 += width

        with (
            tc.tile_pool(name="consts", bufs=1) as cpool,
            tc.tile_pool(name="xin", bufs=10) as inpool,
            # shallow on purpose: when stores lag, compute (and with it the
            # load stream) throttles, so write bytes move during the mixed
            # phase instead of piling into a queue-limited write-only tail
            tc.tile_pool(name="xout", bufs=4) as outpool,
            tc.tile_pool(name="psum", bufs=8, space="PSUM") as pspool,
        ):
            # tile 0's load carries consts + e0 in one dispatch
            w0 = tiles[0][2]
            first_t = cpool.tile([F, CPAD + T_MAX], _BF16, tag="first")
            nc.sync.dma_start(first_t[:, : CPAD + w0], e_all[:, 0 : CPAD + w0])
            w_e_sb = first_t[:, 0:HALF]
            w_n_sb = first_t[:, HALF : 2 * HALF]
            b_e_sb = first_t[:, 2 * HALF : 2 * HALF + 2].bitcast(_FP32)
            b_n_sb = first_t[:, 2 * HALF + 2 : 2 * HALF + 4].bitcast(_FP32)

            # first h-loads pre-dispatched on scalar's HWDGE ring: two
            # queues dispatch the ramp in parallel. Scalar's first ACT op
            # needs the first matmul anyway, so the dispatches ahead of it
            # in scalar's FIFO don't delay compute.
            h_pre = {}
            for idx in range(_H_SCALAR):
                bb, n0, width = tiles[idx]
                h_t = inpool.tile([F, T_MAX], _BF16, tag="h")
                nc.scalar.dma_start(
                    h_t[:, :width], h_all[:, bass.ds(bb * NS + n0, width)]
                )
                h_pre[idx] = h_t

            for idx, (bb, n0, width) in enumerate(tiles):
                sl = bass.ds(n0, width)
                if idx == 0:
                    e_t = first_t
                    e_off = CPAD
                else:
                    e_t = inpool.tile([F, T_MAX], _BF16, tag="e")
                    e_off = 0
                    nc.sync.dma_start(
                        e_t[:, :width], e_all[:, bass.ds(CPAD + bb * NS + n0, width)]
                    )
                if idx in h_pre:
                    h_t = h_pre.pop(idx)
                else:
                    h_t = inpool.tile([F, T_MAX], _BF16, tag="h")
                    nc.sync.dma_start(
                        h_t[:, :width], h_all[:, bass.ds(bb * NS + n0, width)]
                    )
                o_e = outpool.tile([F, T_MAX], _BF16, tag="oe")
                o_n = outpool.tile([F, T_MAX], _BF16, tag="on")
                # all edge matmuls first, then all node matmuls: fewer
                # weight-buffer alternations on PE
                c0 = 0
                for w in _mm_splits(width):
                    ps_e = pspool.tile([HALF, 512], _FP32, tag="ps")
                    nc.tensor.matmul(
                        ps_e[:, :w], w_e_sb, e_t[:, e_off + c0 : e_off + c0 + w]
                    )
                    nc.scalar.activation(
                        o_e[:, c0 : c0 + w],
                        ps_e[:, :w],
                        relu,
                        bias=b_e_sb,
                    )
                    c0 += w
                # edge-half store from ACT's HWDGE ring: depends only on
                # ACT's own output, so no cross-engine HOL
                nc.scalar.dma_start(out[bb, 0:HALF, sl], o_e[:, :width])
                c0 = 0
                for w in _mm_splits(width):
                    ps_n = pspool.tile([HALF, 512], _FP32, tag="ps")
                    nc.tensor.matmul(ps_n[:, :w], w_n_sb, h_t[:, c0 : c0 + w])
                    nc.vector.tensor_scalar(
                        o_n[:, c0 : c0 + w],
                        ps_n[:, :w],
                        b_n_sb,
                        0.0,
                        alu_add,
                        alu_max,
                    )
                    c0 += w
                # node-half store on SWDGE (gpsimd is otherwise idle)
                nc.gpsimd.dma_start(out[bb, HALF : 2 * HALF, sl], o_n[:, :width])

    nc.compile()
    return nc


def _get_nc():
    global _compiled
    if _compiled is None:
        _compiled = _build()
    return _compiled


def run(h_w, e_vw, W_e, b_e, W_n, b_n, trace=False, **kwargs):
    nc = _get_nc()
    bf16 = ml_dtypes.bfloat16
    h_w16 = np.asarray(h_w, dtype=np.float32).astype(bf16)
    e_vw16 = np.asarray(e_vw, dtype=np.float32).astype(bf16)
    consts = np.zeros((F, CPAD), dtype=bf16)
    consts[:, 0:HALF] = np.asarray(W_e, dtype=np.float32).astype(bf16)
    consts[:, HALF : 2 * HALF] = np.asarray(W_n, dtype=np.float32).astype(bf16)
    c_u16 = consts.view(np.uint16)
    c_u16[:, 2 * HALF : 2 * HALF + 2] = (
        np.asarray(b_e, dtype=np.float32).view(np.uint16).reshape(HALF, 2)
    )
    c_u16[:, 2 * HALF + 2 : 2 * HALF + 4] = (
        np.asarray(b_n, dtype=np.float32).view(np.uint16).reshape(HALF, 2)
    )

    in_maps = []
    for c in range(N_CORES):
        sl = slice(c * NS, (c + 1) * NS)
        # fold batch into the node axis: [B, F, ns] -> [F, B*ns]
        e_fold = (
            e_vw16[:, :, sl].transpose(1, 0, 2).reshape(F, B * NS)
        )
        h_fold = (
            h_w16[:, :, sl].transpose(1, 0, 2).reshape(F, B * NS)
        )
        e_core = np.empty((F, CPAD + B * NS), dtype=bf16)
        e_core[:, :CPAD] = consts
        e_core[:, CPAD:] = e_fold
        in_maps.append(
            {
                "e_all": e_core,
                "h_all": np.ascontiguousarray(h_fold),
            }
        )
    res = run_bass_kernel_spmd(
        nc, in_maps, core_ids=list(range(N_CORES)), trace=trace, **kwargs
    )
    full16 = np.concatenate([res.results[c]["out"] for c in range(N_CORES)], axis=2)
    return full16.astype(np.float32), res


def kernel(h_v=None, h_w=None, e_vw=None, W_e=None, b_e=None, W_n=None, b_n=None):
    full, _ = run(h_w, e_vw, W_e, b_e, W_n, b_n, trace=False)
    return full
